# revision 33
# baseline (speedup 1.0000x reference)
"""Trainium2 Bass kernel for nn_DiagonalVariational.

out[i, d] = m[d] + sqrt(log_diag_L[d]^2 + 1e-6) * eps[i, d]

Sharding: data-parallel over the **d axis** — each of the 8 cores gets a
[2048, 2048] column slice of eps/out plus the matching [2048] slices of
m and log_diag_L. Column sharding (instead of n_sample sharding) makes
the per-core [d]-vector broadcast 8x smaller, small enough to do with a
stride-0 DMA read from DRAM (~2 MB extra HBM traffic, ~6 us) instead of
gpsimd partition_broadcast (which measures ~3x slower than its cost
model on HW and dominated n_sample-sharded variants).

Per-core kernel: partition = sample row, free = local d, 16 slabs of
[128, 2048] (1 MB DMAs). scale = sqrt(l^2 + jitter) (one Newton step —
the ACT Sqrt table is only ~1e-6 relative) is computed in a [128, 16]
view and staged through a DRAM scratch so the broadcast can re-read it
row-wise. Loads ride the SP HWDGE ring, stores the ACT ring, so stores
never head-of-line block the eps load stream. Each tile takes two fp32
tensor_tensor ops (mul scale_b, add m_b) on the vector engine; the tail
slab is split into quarter-width pieces so the kernel doesn't end on a
full-width compute+store chain.
"""

import sys

sys.path.insert(0, "/opt/trn_rl_repo")

import numpy as np

D = 16384
N_SAMPLE = 2048
N_CORES = 8
D_LOCAL = D // N_CORES  # 2048
P = 128
JITTER = 1e-6

_CACHE = {}


def _build(
    eps_bufs=8,
    slab_pair=1,
    gpsimd_slabs=0,
    tail_split=4,
    bcast_ring="sync",
    scale_mode="bcast",
    repeat=1,
    setup_in_loop=False,
):
    import contextlib

    import concourse.bacc as bacc
    import concourse.mybir as mybir
    from concourse.tile import TileContext

    DL = D_LOCAL
    n_groups = N_SAMPLE // (P * slab_pair)

    nc = bacc.Bacc("TRN2", target_bir_lowering=False, debug=False, num_devices=N_CORES)

    m_d = nc.dram_tensor("m", (DL,), mybir.dt.float32, kind="ExternalInput").ap()
    l_d = nc.dram_tensor(
        "log_diag_L", (DL,), mybir.dt.float32, kind="ExternalInput"
    ).ap()
    eps_d = nc.dram_tensor(
        "eps", (N_SAMPLE, DL), mybir.dt.float32, kind="ExternalInput"
    ).ap()
    out_d = nc.dram_tensor(
        "out", (N_SAMPLE, DL), mybir.dt.float32, kind="ExternalOutput"
    ).ap()

    with TileContext(nc) as tc:
        with (
            tc.tile_pool(name="setup", bufs=1) as setup_pool,
            tc.tile_pool(name="dram", bufs=1, space="DRAM") as dram_pool,
            tc.tile_pool(name="eps", bufs=eps_bufs) as eps_pool,
        ):
            s_b = setup_pool.tile([P, DL], mybir.dt.float32)
            m_b = setup_pool.tile([P, DL], mybir.dt.float32)

            bcast_eng = {
                "gpsimd": nc.gpsimd,
                "scalar": nc.scalar,
                "sync": nc.sync,
            }[bcast_ring]

            if scale_mode == "bcast":
                # Broadcast the raw log_diag_L (no dependencies — the DMA
                # fires immediately, no scratch roundtrip blocking the load
                # FIFO) and compute scale in broadcast form on DVE/ACT
                # slack. Every partition redundantly computes the same
                # values; ~12 us of otherwise-idle engine time.
                x_b = setup_pool.tile([P, DL], mybir.dt.float32)
                r_b = setup_pool.tile([P, DL], mybir.dt.float32)

                def setup():
                    bcast_eng.dma_start(
                        out=s_b[:], in_=l_d[None, :].to_broadcast((P, DL))
                    )
                    bcast_eng.dma_start(
                        out=m_b[:], in_=m_d[None, :].to_broadcast((P, DL))
                    )
                    nc.vector.tensor_mul(out=x_b[:], in0=s_b[:], in1=s_b[:])
                    nc.vector.tensor_scalar_add(
                        out=x_b[:], in0=x_b[:], scalar1=JITTER
                    )
                    nc.scalar.activation(
                        s_b[:], x_b[:], mybir.ActivationFunctionType.Sqrt
                    )
                    # one Newton step: s = (s0 + x/s0)/2 — the ACT Sqrt
                    # table is only ~1e-6 relative
                    nc.vector.reciprocal(out=r_b[:], in_=s_b[:])
                    nc.vector.tensor_mul(out=r_b[:], in0=r_b[:], in1=x_b[:])
                    nc.vector.tensor_add(out=s_b[:], in0=s_b[:], in1=r_b[:])
                    nc.vector.tensor_scalar_mul(
                        out=s_b[:], in0=s_b[:], scalar1=0.5
                    )

            else:
                W = DL // P
                l_t = setup_pool.tile([P, W], mybir.dt.float32)
                sq_t = setup_pool.tile([P, W], mybir.dt.float32)
                scale_t = setup_pool.tile([P, W], mybir.dt.float32)
                rcp_t = setup_pool.tile([P, W], mybir.dt.float32)
                scratch = dram_pool.tile([P, W], mybir.dt.float32)
                scratch_flat = scratch[:].rearrange("a b -> (a b)")

                def setup():
                    bcast_eng.dma_start(
                        out=m_b[:], in_=m_d[None, :].to_broadcast((P, DL))
                    )
                    nc.sync.dma_start(
                        out=l_t[:], in_=l_d.rearrange("(a b) -> a b", b=W)
                    )
                    nc.vector.tensor_mul(out=sq_t[:], in0=l_t[:], in1=l_t[:])
                    nc.vector.tensor_scalar_add(
                        out=sq_t[:], in0=sq_t[:], scalar1=JITTER
                    )
                    nc.scalar.activation(
                        scale_t[:], sq_t[:], mybir.ActivationFunctionType.Sqrt
                    )
                    nc.vector.reciprocal(out=rcp_t[:], in_=scale_t[:])
                    nc.vector.tensor_mul(out=rcp_t[:], in0=rcp_t[:], in1=sq_t[:])
                    nc.vector.tensor_add(out=scale_t[:], in0=scale_t[:], in1=rcp_t[:])
                    nc.vector.tensor_scalar_mul(
                        out=scale_t[:], in0=scale_t[:], scalar1=0.5
                    )
                    nc.scalar.dma_start(out=scratch[:], in_=scale_t[:])
                    bcast_eng.dma_start(
                        out=s_b[:], in_=scratch_flat[None, :].to_broadcast((P, DL))
                    )

            if not setup_in_loop:
                setup()

            loop_ctx = (
                tc.For_i(0, repeat, 1) if repeat > 1 else contextlib.nullcontext()
            )
            with loop_ctx:
                if setup_in_loop:
                    setup()
                gp_set = set(range(1, 1 + gpsimd_slabs))
                for g in range(n_groups):
                    rs = slice(g * P * slab_pair, (g + 1) * P * slab_pair)
                    src = eps_d[rs, :].rearrange("(s p) d -> p s d", p=P)
                    dst = out_d[rs, :].rearrange("(s p) d -> p s d", p=P)
                    t = eps_pool.tile([P, slab_pair, DL], mybir.dt.float32, tag="t")
                    eng = nc.gpsimd if g in gp_set else nc.vector
                    last = g == n_groups - 1
                    strips = tail_split if (last and tail_split > 1) else 1
                    step = DL // strips
                    for j in range(0, DL, step):
                        js = slice(j, j + step)
                        # 3D tensor ops: in1 broadcasts along the middle
                        # (slab) axis with stride 0
                        sv = s_b[:, None, js].to_broadcast((P, slab_pair, step))
                        mv = m_b[:, None, js].to_broadcast((P, slab_pair, step))
                        nc.sync.dma_start(out=t[:, :, js], in_=src[:, :, js])
                        eng.tensor_mul(out=t[:, :, js], in0=t[:, :, js], in1=sv)
                        eng.tensor_add(out=t[:, :, js], in0=t[:, :, js], in1=mv)
                        nc.scalar.dma_start(out=dst[:, :, js], in_=t[:, :, js])

    nc.compile()
    return nc


def _get_nc():
    if "nc" not in _CACHE:
        _CACHE["nc"] = _build()
    return _CACHE["nc"]


def _shard_inputs(m, log_diag_L, eps):
    m = np.ascontiguousarray(m, dtype=np.float32)
    log_diag_L = np.ascontiguousarray(log_diag_L, dtype=np.float32)
    eps = np.ascontiguousarray(eps, dtype=np.float32)
    return [
        {
            "m": m[i * D_LOCAL : (i + 1) * D_LOCAL],
            "log_diag_L": log_diag_L[i * D_LOCAL : (i + 1) * D_LOCAL],
            "eps": np.ascontiguousarray(eps[:, i * D_LOCAL : (i + 1) * D_LOCAL]),
        }
        for i in range(N_CORES)
    ]


def _gather_out(shards):
    return np.concatenate(list(shards), axis=1)


def kernel(m, log_diag_L, eps, **run_kwargs):
    from concourse import bass_utils

    nc = _get_nc()
    in_maps = _shard_inputs(m, log_diag_L, eps)
    res = bass_utils.run_bass_kernel_spmd(
        nc, in_maps, core_ids=list(range(N_CORES)), **run_kwargs
    )
    out = _gather_out(r["out"] for r in res.results)
    if run_kwargs:
        _CACHE["last_results"] = res
    return out


# revision 34
# speedup vs baseline: 1.0515x; 1.0515x over previous
"""Trainium2 Bass kernel for nn_DiagonalVariational.

out[i, d] = m[d] + sqrt(log_diag_L[d]^2 + 1e-6) * eps[i, d]

Sharding: data-parallel over the **d axis** — each of the 8 cores gets a
[2048, 2048] column slice of eps/out plus the matching [2048] slices of
m and log_diag_L. Column sharding (instead of n_sample sharding) makes
the per-core [d]-vector broadcast 8x smaller, small enough to do with a
stride-0 DMA read from DRAM (~2 MB extra HBM traffic, ~6 us) instead of
gpsimd partition_broadcast (which measures ~3x slower than its cost
model on HW and dominated n_sample-sharded variants).

Per-core kernel: partition = sample row, free = local d, 16 slabs of
[128, 2048] (1 MB DMAs). scale = sqrt(l^2 + jitter) (one Newton step —
the ACT Sqrt table is only ~1e-6 relative) is computed in a [128, 16]
view and staged through a DRAM scratch so the broadcast can re-read it
row-wise. Loads ride the SP HWDGE ring, stores the ACT ring, so stores
never head-of-line block the eps load stream. Each tile takes two fp32
tensor_tensor ops (mul scale_b, add m_b) on the vector engine; the tail
slab is split into quarter-width pieces so the kernel doesn't end on a
full-width compute+store chain.
"""

import sys

sys.path.insert(0, "/opt/trn_rl_repo")

import numpy as np

D = 16384
N_SAMPLE = 2048
N_CORES = 8
D_LOCAL = D // N_CORES  # 2048
P = 128
JITTER = 1e-6

_CACHE = {}


def _build(
    eps_bufs=8,
    slab_pair=1,
    gpsimd_slabs=0,
    tail_split=4,
    bcast_ring="sync",
    scale_mode="scratch",
    repeat=1,
    setup_in_loop=False,
):
    import contextlib

    import concourse.bacc as bacc
    import concourse.mybir as mybir
    from concourse.tile import TileContext

    DL = D_LOCAL
    n_groups = N_SAMPLE // (P * slab_pair)

    nc = bacc.Bacc("TRN2", target_bir_lowering=False, debug=False, num_devices=N_CORES)

    m_d = nc.dram_tensor("m", (DL,), mybir.dt.float32, kind="ExternalInput").ap()
    l_d = nc.dram_tensor(
        "log_diag_L", (DL,), mybir.dt.float32, kind="ExternalInput"
    ).ap()
    eps_d = nc.dram_tensor(
        "eps", (N_SAMPLE, DL), mybir.dt.float32, kind="ExternalInput"
    ).ap()
    out_d = nc.dram_tensor(
        "out", (N_SAMPLE, DL), mybir.dt.float32, kind="ExternalOutput"
    ).ap()

    with TileContext(nc) as tc:
        with (
            tc.tile_pool(name="setup", bufs=1) as setup_pool,
            tc.tile_pool(name="dram", bufs=1, space="DRAM") as dram_pool,
            tc.tile_pool(name="eps", bufs=eps_bufs) as eps_pool,
        ):
            s_b = setup_pool.tile([P, DL], mybir.dt.float32)
            m_b = setup_pool.tile([P, DL], mybir.dt.float32)

            bcast_eng = {
                "gpsimd": nc.gpsimd,
                "scalar": nc.scalar,
                "sync": nc.sync,
            }[bcast_ring]

            if scale_mode == "bcast":
                # Broadcast the raw log_diag_L (no dependencies — the DMA
                # fires immediately, no scratch roundtrip blocking the load
                # FIFO) and compute scale in broadcast form on DVE/ACT
                # slack. Every partition redundantly computes the same
                # values; ~12 us of otherwise-idle engine time.
                x_b = setup_pool.tile([P, DL], mybir.dt.float32)
                r_b = setup_pool.tile([P, DL], mybir.dt.float32)

                def setup():
                    bcast_eng.dma_start(
                        out=s_b[:], in_=l_d[None, :].to_broadcast((P, DL))
                    )
                    bcast_eng.dma_start(
                        out=m_b[:], in_=m_d[None, :].to_broadcast((P, DL))
                    )
                    nc.vector.tensor_mul(out=x_b[:], in0=s_b[:], in1=s_b[:])
                    nc.vector.tensor_scalar_add(
                        out=x_b[:], in0=x_b[:], scalar1=JITTER
                    )
                    nc.scalar.activation(
                        s_b[:], x_b[:], mybir.ActivationFunctionType.Sqrt
                    )
                    # one Newton step: s = (s0 + x/s0)/2 — the ACT Sqrt
                    # table is only ~1e-6 relative
                    nc.vector.reciprocal(out=r_b[:], in_=s_b[:])
                    nc.vector.tensor_mul(out=r_b[:], in0=r_b[:], in1=x_b[:])
                    nc.vector.tensor_add(out=s_b[:], in0=s_b[:], in1=r_b[:])
                    nc.vector.tensor_scalar_mul(
                        out=s_b[:], in0=s_b[:], scalar1=0.5
                    )

            else:
                W = DL // P
                l_t = setup_pool.tile([P, W], mybir.dt.float32)
                sq_t = setup_pool.tile([P, W], mybir.dt.float32)
                scale_t = setup_pool.tile([P, W], mybir.dt.float32)
                rcp_t = setup_pool.tile([P, W], mybir.dt.float32)
                scratch = dram_pool.tile([P, W], mybir.dt.float32)
                scratch_flat = scratch[:].rearrange("a b -> (a b)")

                def setup():
                    bcast_eng.dma_start(
                        out=m_b[:], in_=m_d[None, :].to_broadcast((P, DL))
                    )
                    nc.sync.dma_start(
                        out=l_t[:], in_=l_d.rearrange("(a b) -> a b", b=W)
                    )
                    nc.vector.tensor_mul(out=sq_t[:], in0=l_t[:], in1=l_t[:])
                    nc.vector.tensor_scalar_add(
                        out=sq_t[:], in0=sq_t[:], scalar1=JITTER
                    )
                    nc.scalar.activation(
                        scale_t[:], sq_t[:], mybir.ActivationFunctionType.Sqrt
                    )
                    nc.vector.reciprocal(out=rcp_t[:], in_=scale_t[:])
                    nc.vector.tensor_mul(out=rcp_t[:], in0=rcp_t[:], in1=sq_t[:])
                    nc.vector.tensor_add(out=scale_t[:], in0=scale_t[:], in1=rcp_t[:])
                    nc.vector.tensor_scalar_mul(
                        out=scale_t[:], in0=scale_t[:], scalar1=0.5
                    )
                    nc.scalar.dma_start(out=scratch[:], in_=scale_t[:])
                    bcast_eng.dma_start(
                        out=s_b[:], in_=scratch_flat[None, :].to_broadcast((P, DL))
                    )

            if not setup_in_loop:
                setup()

            loop_ctx = (
                tc.For_i(0, repeat, 1) if repeat > 1 else contextlib.nullcontext()
            )
            with loop_ctx:
                if setup_in_loop:
                    setup()
                gp_set = set(range(1, 1 + gpsimd_slabs))
                for g in range(n_groups):
                    rs = slice(g * P * slab_pair, (g + 1) * P * slab_pair)
                    src = eps_d[rs, :].rearrange("(s p) d -> p s d", p=P)
                    dst = out_d[rs, :].rearrange("(s p) d -> p s d", p=P)
                    t = eps_pool.tile([P, slab_pair, DL], mybir.dt.float32, tag="t")
                    eng = nc.gpsimd if g in gp_set else nc.vector
                    last = g == n_groups - 1
                    strips = tail_split if (last and tail_split > 1) else 1
                    step = DL // strips
                    for j in range(0, DL, step):
                        js = slice(j, j + step)
                        # 3D tensor ops: in1 broadcasts along the middle
                        # (slab) axis with stride 0
                        sv = s_b[:, None, js].to_broadcast((P, slab_pair, step))
                        mv = m_b[:, None, js].to_broadcast((P, slab_pair, step))
                        nc.sync.dma_start(out=t[:, :, js], in_=src[:, :, js])
                        eng.tensor_mul(out=t[:, :, js], in0=t[:, :, js], in1=sv)
                        eng.tensor_add(out=t[:, :, js], in0=t[:, :, js], in1=mv)
                        nc.scalar.dma_start(out=dst[:, :, js], in_=t[:, :, js])

    nc.compile()
    return nc


def _get_nc():
    if "nc" not in _CACHE:
        _CACHE["nc"] = _build()
    return _CACHE["nc"]


def _shard_inputs(m, log_diag_L, eps):
    m = np.ascontiguousarray(m, dtype=np.float32)
    log_diag_L = np.ascontiguousarray(log_diag_L, dtype=np.float32)
    eps = np.ascontiguousarray(eps, dtype=np.float32)
    return [
        {
            "m": m[i * D_LOCAL : (i + 1) * D_LOCAL],
            "log_diag_L": log_diag_L[i * D_LOCAL : (i + 1) * D_LOCAL],
            "eps": np.ascontiguousarray(eps[:, i * D_LOCAL : (i + 1) * D_LOCAL]),
        }
        for i in range(N_CORES)
    ]


def _gather_out(shards):
    return np.concatenate(list(shards), axis=1)


def kernel(m, log_diag_L, eps, **run_kwargs):
    from concourse import bass_utils

    nc = _get_nc()
    in_maps = _shard_inputs(m, log_diag_L, eps)
    res = bass_utils.run_bass_kernel_spmd(
        nc, in_maps, core_ids=list(range(N_CORES)), **run_kwargs
    )
    out = _gather_out(r["out"] for r in res.results)
    if run_kwargs:
        _CACHE["last_results"] = res
    return out


# revision 40
# speedup vs baseline: 1.0944x; 1.0408x over previous
"""Trainium2 Bass kernel for nn_DiagonalVariational.

out[i, d] = m[d] + sqrt(log_diag_L[d]^2 + 1e-6) * eps[i, d]

Sharding: data-parallel over the **d axis** — each of the 8 cores gets a
[2048, 2048] column slice of eps/out plus the matching [2048] slices of
m and log_diag_L. Column sharding (instead of n_sample sharding) makes
the per-core [d]-vector broadcast 8x smaller, small enough to do with a
stride-0 DMA read from DRAM (~2 MB extra HBM traffic, ~6 us) instead of
gpsimd partition_broadcast (which measures ~3x slower than its cost
model on HW and dominated n_sample-sharded variants).

Per-core kernel: partition = sample row, free = local d, 16 slabs of
[128, 2048] (1 MB DMAs). scale = sqrt(l^2 + jitter) (one Newton step —
the ACT Sqrt table is only ~1e-6 relative) is computed in a [128, 16]
view and staged through a DRAM scratch so the broadcast can re-read it
row-wise. Loads ride the SP HWDGE ring, stores the ACT ring, so stores
never head-of-line block the eps load stream. Each tile takes two fp32
tensor_tensor ops (mul scale_b, add m_b) on the vector engine; the tail
slab is split into quarter-width pieces so the kernel doesn't end on a
full-width compute+store chain.
"""

import sys

sys.path.insert(0, "/opt/trn_rl_repo")

import numpy as np

D = 16384
N_SAMPLE = 2048
N_CORES = 8
D_LOCAL = D // N_CORES  # 2048
P = 128
JITTER = 1e-6

_CACHE = {}


def _build(
    eps_bufs=8,
    slab_pair=1,
    gpsimd_slabs=0,
    tail_split=4,
    bcast_ring="sync",
    scale_mode="scratch",
    bcast_transport="dma",
    repeat=1,
    setup_in_loop=False,
):
    import contextlib

    import concourse.bacc as bacc
    import concourse.mybir as mybir
    from concourse.tile import TileContext

    DL = D_LOCAL
    n_groups = N_SAMPLE // (P * slab_pair)

    nc = bacc.Bacc("TRN2", target_bir_lowering=False, debug=False, num_devices=N_CORES)

    m_d = nc.dram_tensor("m", (DL,), mybir.dt.float32, kind="ExternalInput").ap()
    l_d = nc.dram_tensor(
        "log_diag_L", (DL,), mybir.dt.float32, kind="ExternalInput"
    ).ap()
    eps_d = nc.dram_tensor(
        "eps", (N_SAMPLE, DL), mybir.dt.float32, kind="ExternalInput"
    ).ap()
    out_d = nc.dram_tensor(
        "out", (N_SAMPLE, DL), mybir.dt.float32, kind="ExternalOutput"
    ).ap()

    with TileContext(nc) as tc:
        with (
            tc.tile_pool(name="setup", bufs=1) as setup_pool,
            tc.tile_pool(name="dram", bufs=1, space="DRAM") as dram_pool,
            tc.tile_pool(name="eps", bufs=eps_bufs) as eps_pool,
        ):
            s_b = setup_pool.tile([P, DL], mybir.dt.float32)
            m_b = setup_pool.tile([P, DL], mybir.dt.float32)

            bcast_eng = {
                "gpsimd": nc.gpsimd,
                "scalar": nc.scalar,
                "sync": nc.sync,
            }[bcast_ring]

            if scale_mode == "bcast":
                # Broadcast the raw log_diag_L (no dependencies — the DMA
                # fires immediately, no scratch roundtrip blocking the load
                # FIFO) and compute scale in broadcast form on DVE/ACT
                # slack. Every partition redundantly computes the same
                # values; ~12 us of otherwise-idle engine time.
                x_b = setup_pool.tile([P, DL], mybir.dt.float32)
                r_b = setup_pool.tile([P, DL], mybir.dt.float32)

                def setup():
                    bcast_eng.dma_start(
                        out=s_b[:], in_=l_d[None, :].to_broadcast((P, DL))
                    )
                    bcast_eng.dma_start(
                        out=m_b[:], in_=m_d[None, :].to_broadcast((P, DL))
                    )
                    nc.vector.tensor_mul(out=x_b[:], in0=s_b[:], in1=s_b[:])
                    nc.vector.tensor_scalar_add(
                        out=x_b[:], in0=x_b[:], scalar1=JITTER
                    )
                    nc.scalar.activation(
                        s_b[:], x_b[:], mybir.ActivationFunctionType.Sqrt
                    )
                    # one Newton step: s = (s0 + x/s0)/2 — the ACT Sqrt
                    # table is only ~1e-6 relative
                    nc.vector.reciprocal(out=r_b[:], in_=s_b[:])
                    nc.vector.tensor_mul(out=r_b[:], in0=r_b[:], in1=x_b[:])
                    nc.vector.tensor_add(out=s_b[:], in0=s_b[:], in1=r_b[:])
                    nc.vector.tensor_scalar_mul(
                        out=s_b[:], in0=s_b[:], scalar1=0.5
                    )

                def late_setup():
                    pass

            else:
                W = DL // P
                l_t = setup_pool.tile([P, W], mybir.dt.float32)
                sq_t = setup_pool.tile([P, W], mybir.dt.float32)
                scale_t = setup_pool.tile([P, W], mybir.dt.float32)
                rcp_t = setup_pool.tile([P, W], mybir.dt.float32)
                scratch = dram_pool.tile([P, W], mybir.dt.float32)
                scratch_flat = scratch[:].rearrange("a b -> (a b)")
                if bcast_transport == "pb":
                    s_row = setup_pool.tile([1, DL], mybir.dt.float32)
                    m_row = setup_pool.tile([1, DL], mybir.dt.float32)

                def setup():
                    if bcast_transport == "pb":
                        # rows ride the ACT ring (m_row dep-free; s_row
                        # chained right behind the scratch store), then
                        # gpsimd replicates across partitions — zero bytes
                        # on the DMA stream for the [128, DL] broadcasts
                        nc.scalar.dma_start(out=m_row[:], in_=m_d[None, :])
                    else:
                        bcast_eng.dma_start(
                            out=m_b[:], in_=m_d[None, :].to_broadcast((P, DL))
                        )
                    nc.sync.dma_start(
                        out=l_t[:], in_=l_d.rearrange("(a b) -> a b", b=W)
                    )
                    nc.vector.tensor_mul(out=sq_t[:], in0=l_t[:], in1=l_t[:])
                    nc.vector.tensor_scalar_add(
                        out=sq_t[:], in0=sq_t[:], scalar1=JITTER
                    )
                    nc.scalar.activation(
                        scale_t[:], sq_t[:], mybir.ActivationFunctionType.Sqrt
                    )
                    nc.vector.reciprocal(out=rcp_t[:], in_=scale_t[:])
                    nc.vector.tensor_mul(out=rcp_t[:], in0=rcp_t[:], in1=sq_t[:])
                    nc.vector.tensor_add(out=scale_t[:], in0=scale_t[:], in1=rcp_t[:])
                    nc.vector.tensor_scalar_mul(
                        out=scale_t[:], in0=scale_t[:], scalar1=0.5
                    )
                    nc.scalar.dma_start(out=scratch[:], in_=scale_t[:])
                    if bcast_transport == "pb":
                        nc.gpsimd.partition_broadcast(m_b[:], m_row[:])
                    else:
                        bcast_eng.dma_start(
                            out=s_b[:],
                            in_=scratch_flat[None, :].to_broadcast((P, DL)),
                        )

            def late_setup():
                # issued between early eps loads: by now the scratch write
                # has landed, so this trigger fires without blocking the
                # load FIFO, and gpsimd replicates off the DMA stream
                if bcast_transport == "pb":
                    nc.sync.dma_start(out=s_row[:], in_=scratch_flat[None, :])
                    nc.gpsimd.partition_broadcast(s_b[:], s_row[:])

            if not setup_in_loop:
                setup()

            loop_ctx = (
                tc.For_i(0, repeat, 1) if repeat > 1 else contextlib.nullcontext()
            )
            with loop_ctx:
                if setup_in_loop:
                    setup()
                gp_set = set(range(1, 1 + gpsimd_slabs))

                def group_aps(g):
                    rs = slice(g * P * slab_pair, (g + 1) * P * slab_pair)
                    src = eps_d[rs, :].rearrange("(s p) d -> p s d", p=P)
                    dst = out_d[rs, :].rearrange("(s p) d -> p s d", p=P)
                    return src, dst

                def load_group(g):
                    src, _ = group_aps(g)
                    t = eps_pool.tile([P, slab_pair, DL], mybir.dt.float32, tag="t")
                    nc.sync.dma_start(out=t[:], in_=src)
                    return t

                def compute_group(g, t):
                    _, dst = group_aps(g)
                    eng = nc.gpsimd if g in gp_set else nc.vector
                    last = g == n_groups - 1
                    strips = tail_split if (last and tail_split > 1) else 1
                    step = DL // strips
                    for j in range(0, DL, step):
                        js = slice(j, j + step)
                        # 3D tensor ops: in1 broadcasts along the middle
                        # (slab) axis with stride 0
                        sv = s_b[:, None, js].to_broadcast((P, slab_pair, step))
                        mv = m_b[:, None, js].to_broadcast((P, slab_pair, step))
                        eng.tensor_mul(out=t[:, :, js], in0=t[:, :, js], in1=sv)
                        eng.tensor_add(out=t[:, :, js], in0=t[:, :, js], in1=mv)
                        nc.scalar.dma_start(out=dst[:, :, js], in_=t[:, :, js])

                # first few groups load before late_setup (their loads hide
                # the s_row + broadcast latency); their computes come after
                # it in program order so the s_b dependency is tracked
                n_early = min(3, n_groups)
                early = [(g, load_group(g)) for g in range(n_early)]
                late_setup()
                for g, t in early:
                    compute_group(g, t)
                for g in range(n_early, n_groups):
                    t = load_group(g)
                    compute_group(g, t)

    nc.compile()
    return nc


def _get_nc():
    if "nc" not in _CACHE:
        _CACHE["nc"] = _build()
    return _CACHE["nc"]


def _shard_inputs(m, log_diag_L, eps):
    m = np.ascontiguousarray(m, dtype=np.float32)
    log_diag_L = np.ascontiguousarray(log_diag_L, dtype=np.float32)
    eps = np.ascontiguousarray(eps, dtype=np.float32)
    return [
        {
            "m": m[i * D_LOCAL : (i + 1) * D_LOCAL],
            "log_diag_L": log_diag_L[i * D_LOCAL : (i + 1) * D_LOCAL],
            "eps": np.ascontiguousarray(eps[:, i * D_LOCAL : (i + 1) * D_LOCAL]),
        }
        for i in range(N_CORES)
    ]


def _gather_out(shards):
    return np.concatenate(list(shards), axis=1)


def kernel(m, log_diag_L, eps, **run_kwargs):
    from concourse import bass_utils

    nc = _get_nc()
    in_maps = _shard_inputs(m, log_diag_L, eps)
    res = bass_utils.run_bass_kernel_spmd(
        nc, in_maps, core_ids=list(range(N_CORES)), **run_kwargs
    )
    out = _gather_out(r["out"] for r in res.results)
    if run_kwargs:
        _CACHE["last_results"] = res
    return out
